# revision 2
# baseline (speedup 1.0000x reference)
"""LoRA-linear Trainium2 Bass kernel (bf16 in/out, chunk-streamed, pipelined).

Computes, for T adapters: out[t] = x @ W.T + (x @ A_t.T) @ B_t.T + bias
Output: [T, B, S, Dout] float32 (stored bf16 on-device, widened on host).

Sharding: data-parallel over tokens across 8 NeuronCores (2048 tokens/core);
W/bias/selected-LoRA replicated. All matmul inputs are bf16 (host cast);
accumulation is fp32 in PSUM. The output is written to HBM as bf16 — the
dominant HBM stream halves (32 MB -> 16 MB per core), moving the kernel from
DMA-bound to PE-bound; bf16 store rounding (~2^-9 relative) is far inside
the 2e-2 absmax-relative gate.

Per-core schedule, chunk-major over 4 token-chunks of 512:
  prologue  x-chunk0 (1 MB) and W (2 MB) stream in while warm-up matmuls
            hold the HAM clock gate at 2.4 GHz; phase1(c0) runs as soon as
            chunk0 lands (~3us) instead of waiting for all of x (~18us).
  phase1(c) lowT[32t+j, tok] = sum_d A_t[j,d] x[tok,d]   (8 k-matmuls,
            evacuated to SBUF as bf16 by ScalarE)
  base(c,m) W[m-tile] @ x_c.T -> PSUM (8 k-matmuls); ScalarE evacuates
            with the per-partition bias folded in (fp32 in SBUF)
  delta     per t: B_t.T[:, m-tile].T-contract lowT_t — 4 row-group
            matmuls at tile_position (32t, 0), concurrent in the PE array
  add/store VectorE adds base+delta writing bf16 directly; per-(c,m,t)
            contiguous 128 KB stores.
Deltas for (c, m-1) are emitted after base (c, m) so the PE never waits on
VectorE; phase1(c+1) is slotted mid-chunk to stay off the critical path.
"""

import sys

if "/opt/trn_rl_repo" not in sys.path:
    sys.path.insert(0, "/opt/trn_rl_repo")

from contextlib import ExitStack

import ml_dtypes
import numpy as np

import concourse.bacc as bacc
import concourse.bass as bass
import concourse.mybir as mybir
import concourse.tile as tile
from concourse import bass_utils

# Problem constants (hardcoded per spec).
B, S, DIN, DOUT, R, NL, T = 4, 4096, 1024, 1024, 16, 8, 4
NCORES = 8
NTOK = B * S                 # 16384
CTOK = NTOK // NCORES        # 2048 tokens per core
KT = DIN // 128              # 8 k-tiles
MT = DOUT // 128             # 8 dout-tiles
CH = 4                       # token chunks per core
CW = CTOK // CH              # 512 tokens per chunk

WARM1 = 20                   # warm-up matmuls before phase1(c0)
WARM2 = 40                   # warm-up matmuls between phase1(c0) and base

F32 = mybir.dt.float32
BF16 = mybir.dt.bfloat16
NPBF16 = ml_dtypes.bfloat16


def _build_program():
    nc = bacc.Bacc("TRN2", target_bir_lowering=False, debug=False,
                   num_devices=NCORES)

    xc = nc.dram_tensor("xc", [CH, DIN, CW], BF16, kind="ExternalInput").ap()
    wt = nc.dram_tensor("wt", [DIN, DOUT], BF16, kind="ExternalInput").ap()
    atp = nc.dram_tensor("atp", [DIN, 128], BF16, kind="ExternalInput").ap()
    btp = nc.dram_tensor("btp", [128, DOUT], BF16, kind="ExternalInput").ap()
    biasc = nc.dram_tensor("biasc", [128, MT], F32, kind="ExternalInput").ap()
    out = nc.dram_tensor("out", [CH, T, MT, 128, CW], BF16,
                         kind="ExternalOutput").ap()

    with tile.TileContext(nc) as tc, ExitStack() as ctx:
        const = ctx.enter_context(tc.tile_pool(name="const", bufs=1))
        lw_pool = ctx.enter_context(tc.tile_pool(name="lw", bufs=2))
        bsb_pool = ctx.enter_context(tc.tile_pool(name="bsb", bufs=3))
        od_pool = ctx.enter_context(tc.tile_pool(name="od", bufs=20))
        bp_ps = ctx.enter_context(tc.tile_pool(name="bp_ps", bufs=2, space="PSUM"))
        ph_ps = ctx.enter_context(tc.tile_pool(name="ph_ps", bufs=2, space="PSUM"))
        dp_ps = ctx.enter_context(tc.tile_pool(name="dp_ps", bufs=4, space="PSUM"))

        # Small tensors on the scalar HWDGE ring (own FIFO, lands ~1us) so
        # warm-up and phase1 can start immediately.
        at_all = const.tile([128, KT * 128], BF16, tag="at")
        nc.scalar.dma_start(at_all.rearrange("p (k r) -> p k r", k=KT),
                            atp.rearrange("(k p) r -> p k r", p=128))
        bt_s = const.tile([128, DOUT], BF16, tag="bt")
        nc.scalar.dma_start(bt_s[:], btp[:, :])
        bias_s = const.tile([128, MT], F32, tag="bias")
        nc.scalar.dma_start(bias_s[:], biasc[:, :])

        # Big loads on the sync ring, in consumption order: x chunk 0 first
        # (phase1 needs only x), then W (base needs it), then chunks 1-3.
        xc_t = []
        for c in range(CH):
            xc_t.append(const.tile([128, KT * CW], BF16, tag=f"xc{c}"))

        def load_chunk(c):
            nc.sync.dma_start(xc_t[c].rearrange("p (k w) -> p k w", k=KT),
                              xc[c].rearrange("(k p) w -> p k w", p=128))

        load_chunk(0)
        wt_all = const.tile([128, KT * DOUT], BF16, tag="wt")
        nc.sync.dma_start(wt_all.rearrange("p (k n) -> p k n", k=KT),
                          wt.rearrange("(k p) n -> p k n", p=128))
        for c in range(1, CH):
            load_chunk(c)

        lwt = {}

        def emit_phase1(c):
            ph = ph_ps.tile([128, CW], F32, tag="ph", name=f"ph{c}")
            for k in range(KT):
                nc.tensor.matmul(
                    ph[:],
                    at_all[:, bass.ts(k, 128)],
                    xc_t[c][:, bass.ts(k, CW)],
                    start=(k == 0), stop=(k == KT - 1),
                )
            t_ = lw_pool.tile([128, CW], BF16, tag="lw", name=f"lw{c}")
            nc.scalar.copy(t_[:], ph[:])
            lwt[c] = t_

        # Warm-up matmuls: the HAM clock gate needs ~3.4us of sustained PE
        # activity to unthrottle 1.2 -> 2.4 GHz; these span the DMA prologue.
        warm = ph_ps.tile([128, CW], F32, tag="ph", name="warm")
        for _ in range(WARM1):
            nc.tensor.matmul(warm[:, 0:128], at_all[:, 0:128], bt_s[:, 0:128],
                             start=True, stop=True)
        emit_phase1(0)
        for _ in range(WARM2):
            nc.tensor.matmul(warm[:, 0:128], at_all[:, 0:128], bt_s[:, 0:128],
                             start=True, stop=True)

        def emit_base(c, m):
            bp = bp_ps.tile([128, CW], F32, tag="bp", name=f"bp{c}_{m}")
            for k in range(KT):
                nc.tensor.matmul(
                    bp[:],
                    wt_all[:, k * DOUT + m * 128:k * DOUT + (m + 1) * 128],
                    xc_t[c][:, bass.ts(k, CW)],
                    start=(k == 0), stop=(k == KT - 1),
                )
            bsb = bsb_pool.tile([128, CW], F32, tag="bsb", name=f"bsb{c}_{m}")
            nc.scalar.activation(
                bsb[:], bp[:],
                mybir.ActivationFunctionType.Identity,
                bias=bias_s[:, m:m + 1],
            )
            return bsb

        def emit_delta(c, m, bsb):
            for t in range(T):
                dp = dp_ps.tile([128, CW], F32, tag="dp", name=f"dp{c}_{m}_{t}")
                nc.tensor.matmul(
                    dp[:],
                    bt_s[32 * t:32 * t + R, bass.ts(m, 128)],
                    lwt[c][32 * t:32 * t + R, :],
                    start=True, stop=True,
                    tile_position=(32 * t, 0),
                )
                od = od_pool.tile([128, CW], BF16, tag="od", name=f"od{c}_{m}_{t}")
                nc.vector.tensor_add(od[:], bsb[:], dp[:])
                nc.sync.dma_start(out[c, t, m, :, :], od[:])

        prev = None
        for c in range(CH):
            for m in range(MT):
                bsb = emit_base(c, m)
                if m == 4 and c + 1 < CH:
                    emit_phase1(c + 1)
                if prev is not None:
                    emit_delta(*prev)
                prev = (c, m, bsb)
        emit_delta(*prev)

    nc.compile()
    return nc


_NC = None


def _get_program():
    global _NC
    if _NC is None:
        _NC = _build_program()
    return _NC


def kernel(**inputs):
    x = np.ascontiguousarray(np.asarray(inputs["x"], dtype=np.float32))
    W = np.asarray(inputs["W"], dtype=np.float32)
    bias_v = np.asarray(inputs["bias"], dtype=np.float32)
    lora_A = np.asarray(inputs["lora_A"], dtype=np.float32)
    lora_B = np.asarray(inputs["lora_B"], dtype=np.float32)
    tuner_index = np.asarray(inputs["tuner_index"]).astype(np.int64)

    assert x.shape == (B, S, DIN) and W.shape == (DOUT, DIN)
    assert tuner_index.shape == (T,)

    A_sel = lora_A[tuner_index]          # [T, R, Din]
    B_sel = lora_B[tuner_index]          # [T, Dout, R]

    toks = x.reshape(NTOK, DIN)
    wt = np.ascontiguousarray(W.T).astype(NPBF16)       # [Din, Dout]
    atp = np.zeros((DIN, 128), NPBF16)
    atp.reshape(DIN, T, 32)[:, :, :R] = A_sel.transpose(2, 0, 1).astype(NPBF16)
    btp = np.zeros((128, DOUT), NPBF16)
    btp.reshape(T, 32, DOUT)[:, :R, :] = B_sel.transpose(0, 2, 1).astype(NPBF16)
    biasc = np.ascontiguousarray(bias_v.reshape(MT, 128).T)   # [128, MT]

    in_maps = []
    for c in range(NCORES):
        xcore = toks[c * CTOK:(c + 1) * CTOK]            # [2048, 1024]
        # [CH, DIN, CW]: chunk-major, d on rows -> each chunk one 1MB DMA
        xch = xcore.reshape(CH, CW, DIN).transpose(0, 2, 1).astype(NPBF16)
        in_maps.append({
            "xc": np.ascontiguousarray(xch),
            "wt": wt,
            "atp": atp,
            "btp": btp,
            "biasc": biasc,
        })

    nc = _get_program()
    res = bass_utils.run_bass_kernel_spmd(nc, in_maps, core_ids=list(range(NCORES)))

    full = np.empty((T, NTOK, DOUT), np.float32)
    for c in range(NCORES):
        o = np.asarray(res.results[c]["out"])   # [CH, T, MT, 128, CW] bf16
        # o[ch, t, m, p, w] -> [t, ch*CW + w, m*128 + p]
        oc = o.transpose(1, 0, 4, 2, 3).reshape(T, CTOK, DOUT)
        full[:, c * CTOK:(c + 1) * CTOK, :] = oc.astype(np.float32)
    return full.reshape(T, B, S, DOUT)


# revision 5
# speedup vs baseline: 1.0312x; 1.0312x over previous
"""LoRA-linear Trainium2 Bass kernel (bf16 in/out, chunk-streamed, pipelined).

Computes, for T adapters: out[t] = x @ W.T + (x @ A_t.T) @ B_t.T + bias
Output: [T, B, S, Dout] float32 (stored bf16 on-device, widened on host).

Sharding: data-parallel over tokens across 8 NeuronCores (2048 tokens/core);
W/bias/selected-LoRA replicated. All matmul inputs are bf16 (host cast);
accumulation is fp32 in PSUM. The output is written to HBM as bf16 — the
dominant HBM stream halves (32 MB -> 16 MB per core), moving the kernel from
DMA-bound to PE-bound; bf16 store rounding (~2^-9 relative) is far inside
the 2e-2 absmax-relative gate.

Per-core schedule, chunk-major over 4 token-chunks of 512:
  prologue  x-chunk0 (1 MB) and W (2 MB) stream in while warm-up matmuls
            hold the HAM clock gate at 2.4 GHz; phase1(c0) runs as soon as
            chunk0 lands (~3us) instead of waiting for all of x (~18us).
  phase1(c) lowT[32t+j, tok] = sum_d A_t[j,d] x[tok,d]   (8 k-matmuls,
            evacuated to SBUF as bf16 by ScalarE)
  base(c,m) W[m-tile] @ x_c.T -> PSUM (8 k-matmuls); ScalarE evacuates
            with the per-partition bias folded in (fp32 in SBUF)
  delta     per t: B_t.T[:, m-tile].T-contract lowT_t — 4 row-group
            matmuls at tile_position (32t, 0), concurrent in the PE array
  add/store VectorE adds base+delta writing bf16 directly; per-(c,m,t)
            contiguous 128 KB stores.
Deltas for (c, m-1) are emitted after base (c, m) so the PE never waits on
VectorE; phase1(c+1) is slotted mid-chunk to stay off the critical path.
"""

import sys

if "/opt/trn_rl_repo" not in sys.path:
    sys.path.insert(0, "/opt/trn_rl_repo")

from contextlib import ExitStack

import ml_dtypes
import numpy as np

import concourse.bacc as bacc
import concourse.bass as bass
import concourse.mybir as mybir
import concourse.tile as tile
from concourse import bass_utils

# Problem constants (hardcoded per spec).
B, S, DIN, DOUT, R, NL, T = 4, 4096, 1024, 1024, 16, 8, 4
NCORES = 8
NTOK = B * S                 # 16384
CTOK = NTOK // NCORES        # 2048 tokens per core
KT = DIN // 128              # 8 k-tiles
MT = DOUT // 128             # 8 dout-tiles
CH = 4                       # token chunks per core
CW = CTOK // CH              # 512 tokens per chunk

WARM1 = 20                   # warm-up matmuls before phase1(c0)
WARM2 = 40                   # warm-up matmuls between phase1(c0) and base

F32 = mybir.dt.float32
BF16 = mybir.dt.bfloat16
NPBF16 = ml_dtypes.bfloat16


def _build_program():
    nc = bacc.Bacc("TRN2", target_bir_lowering=False, debug=False,
                   num_devices=NCORES)

    xc = nc.dram_tensor("xc", [CH, DIN, CW], BF16, kind="ExternalInput").ap()
    wt = nc.dram_tensor("wt", [DIN, DOUT], BF16, kind="ExternalInput").ap()
    atp = nc.dram_tensor("atp", [DIN, 128], BF16, kind="ExternalInput").ap()
    btp = nc.dram_tensor("btp", [128, DOUT], BF16, kind="ExternalInput").ap()
    biasc = nc.dram_tensor("biasc", [128, MT], F32, kind="ExternalInput").ap()
    out = nc.dram_tensor("out", [CH, T, MT, 128, CW], BF16,
                         kind="ExternalOutput").ap()

    with tile.TileContext(nc) as tc, ExitStack() as ctx:
        const = ctx.enter_context(tc.tile_pool(name="const", bufs=1))
        lw_pool = ctx.enter_context(tc.tile_pool(name="lw", bufs=2))
        bsb_pool = ctx.enter_context(tc.tile_pool(name="bsb", bufs=3))
        ds_pool = ctx.enter_context(tc.tile_pool(name="ds", bufs=4))
        od_pool = ctx.enter_context(tc.tile_pool(name="od", bufs=20))
        bp_ps = ctx.enter_context(tc.tile_pool(name="bp_ps", bufs=2, space="PSUM"))
        ph_ps = ctx.enter_context(tc.tile_pool(name="ph_ps", bufs=2, space="PSUM"))
        dp_ps = ctx.enter_context(tc.tile_pool(name="dp_ps", bufs=4, space="PSUM"))

        # Small tensors on the scalar HWDGE ring (own FIFO, lands ~1us) so
        # warm-up and phase1 can start immediately.
        at_all = const.tile([128, KT * 128], BF16, tag="at")
        nc.scalar.dma_start(at_all.rearrange("p (k r) -> p k r", k=KT),
                            atp.rearrange("(k p) r -> p k r", p=128))
        bt_s = const.tile([128, DOUT], BF16, tag="bt")
        nc.scalar.dma_start(bt_s[:], btp[:, :])
        bias_s = const.tile([128, MT], F32, tag="bias")
        nc.scalar.dma_start(bias_s[:], biasc[:, :])

        # Big loads on the sync ring, in consumption order: x chunk 0 first
        # (phase1 needs only x), then W (base needs it), then chunks 1-3.
        xc_t = []
        for c in range(CH):
            xc_t.append(const.tile([128, KT * CW], BF16, tag=f"xc{c}",
                                   name=f"xc{c}"))

        def load_chunk(c):
            nc.sync.dma_start(xc_t[c].rearrange("p (k w) -> p k w", k=KT),
                              xc[c].rearrange("(k p) w -> p k w", p=128))

        load_chunk(0)
        wt_all = const.tile([128, KT * DOUT], BF16, tag="wt")
        nc.sync.dma_start(wt_all.rearrange("p (k n) -> p k n", k=KT),
                          wt.rearrange("(k p) n -> p k n", p=128))
        for c in range(1, CH):
            load_chunk(c)

        lwt = {}

        def emit_phase1(c):
            ph = ph_ps.tile([128, CW], F32, tag="ph", name=f"ph{c}")
            for k in range(KT):
                nc.tensor.matmul(
                    ph[:],
                    at_all[:, bass.ts(k, 128)],
                    xc_t[c][:, bass.ts(k, CW)],
                    start=(k == 0), stop=(k == KT - 1),
                )
            t_ = lw_pool.tile([128, CW], BF16, tag="lw", name=f"lw{c}")
            nc.scalar.copy(t_[:], ph[:])
            lwt[c] = t_

        # Warm-up matmuls: the HAM clock gate needs ~3.4us of sustained PE
        # activity to unthrottle 1.2 -> 2.4 GHz; these span the DMA prologue.
        warm = ph_ps.tile([128, CW], F32, tag="ph", name="warm")
        for _ in range(WARM1):
            nc.tensor.matmul(warm[:, 0:128], at_all[:, 0:128], bt_s[:, 0:128],
                             start=True, stop=True)
        emit_phase1(0)
        for _ in range(WARM2):
            nc.tensor.matmul(warm[:, 0:128], at_all[:, 0:128], bt_s[:, 0:128],
                             start=True, stop=True)

        def emit_base(c, m):
            bp = bp_ps.tile([128, CW], F32, tag="bp", name=f"bp{c}_{m}")
            for k in range(KT):
                nc.tensor.matmul(
                    bp[:],
                    wt_all[:, k * DOUT + m * 128:k * DOUT + (m + 1) * 128],
                    xc_t[c][:, bass.ts(k, CW)],
                    start=(k == 0), stop=(k == KT - 1),
                )
            # bf16 base: halves ScalarE evac time and enables 2x bf16 adds.
            bsb = bsb_pool.tile([128, CW], BF16, tag="bsb", name=f"bsb{c}_{m}")
            nc.scalar.activation(
                bsb[:], bp[:],
                mybir.ActivationFunctionType.Identity,
                bias=bias_s[:, m:m + 1],
            )
            return bsb

        def emit_delta(c, m, bsb):
            # DVE tensor_tensor with a PSUM operand runs at 1x (~658ns per
            # 512-wide add); all-bf16 SBUF adds run at 2x (~327ns). Route two
            # of the four deltas through a ScalarE PSUM->SBUF bf16 copy so
            # DVE and ScalarE split the per-element work.
            dps = []
            for t in range(T):
                dp = dp_ps.tile([128, CW], F32, tag="dp", name=f"dp{c}_{m}_{t}")
                nc.tensor.matmul(
                    dp[:],
                    bt_s[32 * t:32 * t + R, bass.ts(m, 128)],
                    lwt[c][32 * t:32 * t + R, :],
                    start=True, stop=True,
                    tile_position=(32 * t, 0),
                )
                dps.append(dp)
            dss = {}
            for t in (2, 3):
                ds = ds_pool.tile([128, CW], BF16, tag="ds", name=f"ds{c}_{m}_{t}")
                nc.scalar.copy(ds[:], dps[t][:])
                dss[t] = ds
            for t in range(T):
                od = od_pool.tile([128, CW], BF16, tag="od", name=f"od{c}_{m}_{t}")
                nc.vector.tensor_add(od[:], bsb[:],
                                     dps[t][:] if t < 2 else dss[t][:])
                nc.sync.dma_start(out[c, t, m, :, :], od[:])

        prev = None
        for c in range(CH):
            for m in range(MT):
                bsb = emit_base(c, m)
                if m == 4 and c + 1 < CH:
                    emit_phase1(c + 1)
                if prev is not None:
                    emit_delta(*prev)
                prev = (c, m, bsb)
        emit_delta(*prev)

    nc.compile()
    return nc


_NC = None


def _get_program():
    global _NC
    if _NC is None:
        _NC = _build_program()
    return _NC


def kernel(**inputs):
    x = np.ascontiguousarray(np.asarray(inputs["x"], dtype=np.float32))
    W = np.asarray(inputs["W"], dtype=np.float32)
    bias_v = np.asarray(inputs["bias"], dtype=np.float32)
    lora_A = np.asarray(inputs["lora_A"], dtype=np.float32)
    lora_B = np.asarray(inputs["lora_B"], dtype=np.float32)
    tuner_index = np.asarray(inputs["tuner_index"]).astype(np.int64)

    assert x.shape == (B, S, DIN) and W.shape == (DOUT, DIN)
    assert tuner_index.shape == (T,)

    A_sel = lora_A[tuner_index]          # [T, R, Din]
    B_sel = lora_B[tuner_index]          # [T, Dout, R]

    toks = x.reshape(NTOK, DIN)
    wt = np.ascontiguousarray(W.T).astype(NPBF16)       # [Din, Dout]
    atp = np.zeros((DIN, 128), NPBF16)
    atp.reshape(DIN, T, 32)[:, :, :R] = A_sel.transpose(2, 0, 1).astype(NPBF16)
    btp = np.zeros((128, DOUT), NPBF16)
    btp.reshape(T, 32, DOUT)[:, :R, :] = B_sel.transpose(0, 2, 1).astype(NPBF16)
    biasc = np.ascontiguousarray(bias_v.reshape(MT, 128).T)   # [128, MT]

    in_maps = []
    for c in range(NCORES):
        xcore = toks[c * CTOK:(c + 1) * CTOK]            # [2048, 1024]
        # [CH, DIN, CW]: chunk-major, d on rows -> each chunk one 1MB DMA
        xch = xcore.reshape(CH, CW, DIN).transpose(0, 2, 1).astype(NPBF16)
        in_maps.append({
            "xc": np.ascontiguousarray(xch),
            "wt": wt,
            "atp": atp,
            "btp": btp,
            "biasc": biasc,
        })

    nc = _get_program()
    res = bass_utils.run_bass_kernel_spmd(nc, in_maps, core_ids=list(range(NCORES)))

    full = np.empty((T, NTOK, DOUT), np.float32)
    for c in range(NCORES):
        o = np.asarray(res.results[c]["out"])   # [CH, T, MT, 128, CW] bf16
        # o[ch, t, m, p, w] -> [t, ch*CW + w, m*128 + p]
        oc = o.transpose(1, 0, 4, 2, 3).reshape(T, CTOK, DOUT)
        full[:, c * CTOK:(c + 1) * CTOK, :] = oc.astype(np.float32)
    return full.reshape(T, B, S, DOUT)


# revision 6
# speedup vs baseline: 1.3180x; 1.2781x over previous
"""LoRA-linear Trainium2 Bass kernel (bf16 in/out, chunk-streamed, pipelined).

Computes, for T adapters: out[t] = x @ W.T + (x @ A_t.T) @ B_t.T + bias
Output: [T, B, S, Dout] float32 (stored bf16 on-device, widened on host).

Sharding: data-parallel over tokens across 8 NeuronCores (2048 tokens/core);
W/bias/selected-LoRA replicated. All matmul inputs are bf16 (host cast);
accumulation is fp32 in PSUM; the output is written to HBM as bf16 (16 MB
per core instead of 32), far inside the 2e-2 absmax-relative gate.

Lessons from the v3 NTFF trace baked in here:
 * Every load/store is a single plain DMA with >=2KB contiguous
   per-partition runs (host pre-packs all layouts). Small or strided
   patterns cost 100s of descriptors; descriptor-gen serializes on the
   issuing engine's queue (~0.7us per 128-descriptor DMA) and tiny runs
   drain far below line rate. v3 spent 90us issuing 128 per-(c,m,t)
   stores; v4 issues 32 per-(c,m) stores of [128, T*512] (4KB/partition).
 * Warm-up matmuls read a GpSimd-memset tile, not a DMA'd tile, so the
   HAM clock-gate ramp (needs ~3.4us of PE activity) runs during the DMA
   prologue instead of after it.
 * W streams in two halves (m-tiles 0-3, 4-7) after x-chunk0 so the first
   base matmul starts ~4us earlier.
 * DVE tensor_tensor with a PSUM operand runs at 1x (658ns per 512-wide
   add); all-bf16 SBUF adds run at 2x (327ns). Per (c,m): deltas t0/t1
   are added straight from PSUM, t2/t3 land in one [128,1024] PSUM tile,
   ScalarE copies it to bf16 SBUF (single fused ACTIVATE), and DVE adds
   both halves at 2x. This balances DVE (~63us) and ScalarE (~61us)
   under the PE's ~76us.

Per-core schedule, chunk-major over 4 token-chunks of 512 tokens:
  phase1(c) lowT[32t+j, tok] = sum_d A_t[j,d] x[tok,d]  (8 k-matmuls)
  base(c,m) W[m-tile] @ x_c.T -> PSUM (8 k-matmuls, 216ns cadence);
            ScalarE evacuates with bias folded in, bf16
  delta     per t: 4 row-group matmuls at tile_position (32t,0),
            concurrent in the PE array (~0.4us for all four)
  add/store DVE adds write bf16 slices of od[128, T*512]; one 512KB store
Deltas for (c, m-1) are emitted after base (c, m) so the PE never waits
on DVE/ScalarE; phase1(c+1) is slotted mid-chunk, off the critical path.
"""

import sys

if "/opt/trn_rl_repo" not in sys.path:
    sys.path.insert(0, "/opt/trn_rl_repo")

from contextlib import ExitStack

import ml_dtypes
import numpy as np

import concourse.bacc as bacc
import concourse.bass as bass
import concourse.mybir as mybir
import concourse.tile as tile
from concourse import bass_utils

# Problem constants (hardcoded per spec).
B, S, DIN, DOUT, R, NL, T = 4, 4096, 1024, 1024, 16, 8, 4
NCORES = 8
NTOK = B * S                 # 16384
CTOK = NTOK // NCORES        # 2048 tokens per core
KT = DIN // 128              # 8 k-tiles
MT = DOUT // 128             # 8 dout-tiles
CH = 4                       # token chunks per core
CW = CTOK // CH              # 512 tokens per chunk

WARM1 = 34                   # warm-up matmuls before phase1(c0)
WARM2 = 6                    # warm-up matmuls between phase1(c0) and base

F32 = mybir.dt.float32
BF16 = mybir.dt.bfloat16
NPBF16 = ml_dtypes.bfloat16


def _build_program():
    nc = bacc.Bacc("TRN2", target_bir_lowering=False, debug=False,
                   num_devices=NCORES)

    # All DRAM layouts are pre-packed on host so every DMA is a plain
    # contiguous [128, n] transfer.
    xc = nc.dram_tensor("xc", [CH, 128, KT * CW], BF16, kind="ExternalInput").ap()
    wt = nc.dram_tensor("wt", [128, MT * KT * 128], BF16, kind="ExternalInput").ap()
    atp = nc.dram_tensor("atp", [128, KT * 128], BF16, kind="ExternalInput").ap()
    btp = nc.dram_tensor("btp", [128, DOUT], BF16, kind="ExternalInput").ap()
    biasc = nc.dram_tensor("biasc", [128, MT], F32, kind="ExternalInput").ap()
    out = nc.dram_tensor("out", [CH, MT, 128, T * CW], BF16,
                         kind="ExternalOutput").ap()

    with tile.TileContext(nc) as tc, ExitStack() as ctx:
        const = ctx.enter_context(tc.tile_pool(name="const", bufs=1))
        lw_pool = ctx.enter_context(tc.tile_pool(name="lw", bufs=2))
        bsb_pool = ctx.enter_context(tc.tile_pool(name="bsb", bufs=3))
        ds_pool = ctx.enter_context(tc.tile_pool(name="ds", bufs=2))
        od_pool = ctx.enter_context(tc.tile_pool(name="od", bufs=4))
        bp_ps = ctx.enter_context(tc.tile_pool(name="bp_ps", bufs=2, space="PSUM"))
        ph_ps = ctx.enter_context(tc.tile_pool(name="ph_ps", bufs=2, space="PSUM"))
        dps_ps = ctx.enter_context(tc.tile_pool(name="dps_ps", bufs=2, space="PSUM"))
        dpd_ps = ctx.enter_context(tc.tile_pool(name="dpd_ps", bufs=1, space="PSUM"))

        # Warm-up source: memset by GpSimd (~6us mark), no DMA dependency.
        wsrc = const.tile([128, 128], BF16, tag="wsrc")
        nc.gpsimd.memset(wsrc[:], 0.0)

        # Small tensors on the scalar HWDGE ring (own FIFO).
        at_all = const.tile([128, KT * 128], BF16, tag="at")
        nc.scalar.dma_start(at_all[:], atp[:, :])
        bt_s = const.tile([128, DOUT], BF16, tag="bt")
        nc.scalar.dma_start(bt_s[:], btp[:, :])
        bias_s = const.tile([128, MT], F32, tag="bias")
        nc.scalar.dma_start(bias_s[:], biasc[:, :])

        # Big loads on the sync ring, in consumption order: x chunk 0
        # (phase1 needs only x), W in two m-halves, then chunks 1-3.
        xc_t = []
        for c in range(CH):
            xc_t.append(const.tile([128, KT * CW], BF16, tag=f"xc{c}",
                                   name=f"xc{c}"))
        wt_all = const.tile([128, MT * KT * 128], BF16, tag="wt")

        nc.sync.dma_start(xc_t[0][:], xc[0])
        half = MT * KT * 128 // 2
        nc.sync.dma_start(wt_all[:, 0:half], wt[:, 0:half])
        nc.sync.dma_start(wt_all[:, half:2 * half], wt[:, half:2 * half])
        for c in range(1, CH):
            nc.sync.dma_start(xc_t[c][:], xc[c])

        lwt = {}

        def emit_phase1(c):
            ph = ph_ps.tile([128, CW], F32, tag="ph", name=f"ph{c}")
            for k in range(KT):
                nc.tensor.matmul(
                    ph[:],
                    at_all[:, bass.ts(k, 128)],
                    xc_t[c][:, bass.ts(k, CW)],
                    start=(k == 0), stop=(k == KT - 1),
                )
            t_ = lw_pool.tile([128, CW], BF16, tag="lw", name=f"lw{c}")
            nc.scalar.copy(t_[:], ph[:])
            lwt[c] = t_

        # Warm-up: the HAM clock gate needs ~3.4us of sustained PE activity
        # to unthrottle 1.2 -> 2.4 GHz; run it on the memset tile while the
        # input DMAs stream.
        warm = ph_ps.tile([128, CW], F32, tag="ph", name="warm")
        for _ in range(WARM1):
            nc.tensor.matmul(warm[:, 0:128], wsrc[:], wsrc[:],
                             start=True, stop=True)
        emit_phase1(0)
        for _ in range(WARM2):
            nc.tensor.matmul(warm[:, 0:128], wsrc[:], wsrc[:],
                             start=True, stop=True)

        def emit_base_mms(c, m):
            bp = bp_ps.tile([128, CW], F32, tag="bp", name=f"bp{c}_{m}")
            for k in range(KT):
                nc.tensor.matmul(
                    bp[:],
                    wt_all[:, m * (KT * 128) + k * 128:
                           m * (KT * 128) + (k + 1) * 128],
                    xc_t[c][:, bass.ts(k, CW)],
                    start=(k == 0), stop=(k == KT - 1),
                )
            return bp

        def emit_evac(c, m, bp):
            bsb = bsb_pool.tile([128, CW], BF16, tag="bsb", name=f"bsb{c}_{m}")
            nc.scalar.activation(
                bsb[:], bp[:],
                mybir.ActivationFunctionType.Identity,
                bias=bias_s[:, m:m + 1],
            )
            return bsb

        def emit_delta(c, m, bsb):
            s0 = dps_ps.tile([128, CW], F32, tag="dps", name=f"dp{c}_{m}_0")
            s1 = dps_ps.tile([128, CW], F32, tag="dps", name=f"dp{c}_{m}_1")
            d23 = dpd_ps.tile([128, 2 * CW], F32, tag="dpd", name=f"dp{c}_{m}_23")
            outs = [s0[:], s1[:], d23[:, 0:CW], d23[:, CW:2 * CW]]
            for t in range(T):
                nc.tensor.matmul(
                    outs[t],
                    bt_s[32 * t:32 * t + R, bass.ts(m, 128)],
                    lwt[c][32 * t:32 * t + R, :],
                    start=True, stop=True,
                    tile_position=(32 * t, 0),
                )
            # Fused ScalarE evacuation of t2/t3 ahead of the base evac in the
            # ACT queue so the (single-buffered) d23 bank frees early.
            ds = ds_pool.tile([128, 2 * CW], BF16, tag="ds", name=f"ds{c}_{m}")
            nc.scalar.copy(ds[:], d23[:])
            od = od_pool.tile([128, T * CW], BF16, tag="od", name=f"od{c}_{m}")
            nc.vector.tensor_add(od[:, 0:CW], bsb[:], s0[:])
            nc.vector.tensor_add(od[:, CW:2 * CW], bsb[:], s1[:])
            nc.vector.tensor_add(od[:, 2 * CW:3 * CW], bsb[:], ds[:, 0:CW])
            nc.vector.tensor_add(od[:, 3 * CW:4 * CW], bsb[:], ds[:, CW:2 * CW])
            nc.sync.dma_start(out[c, m, :, :], od[:])

        prev = None
        for c in range(CH):
            for m in range(MT):
                bp = emit_base_mms(c, m)
                if m == 4 and c + 1 < CH:
                    emit_phase1(c + 1)
                if prev is not None:
                    emit_delta(*prev)
                bsb = emit_evac(c, m, bp)
                prev = (c, m, bsb)
        emit_delta(*prev)

    nc.compile()
    return nc


_NC = None


def _get_program():
    global _NC
    if _NC is None:
        _NC = _build_program()
    return _NC


def kernel(**inputs):
    x = np.ascontiguousarray(np.asarray(inputs["x"], dtype=np.float32))
    W = np.asarray(inputs["W"], dtype=np.float32)
    bias_v = np.asarray(inputs["bias"], dtype=np.float32)
    lora_A = np.asarray(inputs["lora_A"], dtype=np.float32)
    lora_B = np.asarray(inputs["lora_B"], dtype=np.float32)
    tuner_index = np.asarray(inputs["tuner_index"]).astype(np.int64)

    assert x.shape == (B, S, DIN) and W.shape == (DOUT, DIN)
    assert tuner_index.shape == (T,)

    A_sel = lora_A[tuner_index]          # [T, R, Din]
    B_sel = lora_B[tuner_index]          # [T, Dout, R]

    toks = x.reshape(NTOK, DIN)
    # wt[p, m, k, n] = W[m*128+n, k*128+p]
    wt = np.ascontiguousarray(
        W.reshape(MT, 128, KT, 128).transpose(3, 0, 2, 1)
    ).astype(NPBF16).reshape(128, MT * KT * 128)
    # atp_flat[d, 32t+j] = A_sel[t, j, d]; then [p, k, j] = [k*128+p, j]
    atp_flat = np.zeros((DIN, 128), np.float32)
    atp_flat.reshape(DIN, T, 32)[:, :, :R] = A_sel.transpose(2, 0, 1)
    atp = np.ascontiguousarray(
        atp_flat.reshape(KT, 128, 128).transpose(1, 0, 2)
    ).astype(NPBF16).reshape(128, KT * 128)
    btp = np.zeros((128, DOUT), NPBF16)
    btp.reshape(T, 32, DOUT)[:, :R, :] = B_sel.transpose(0, 2, 1).astype(NPBF16)
    biasc = np.ascontiguousarray(bias_v.reshape(MT, 128).T)   # [128, MT]

    in_maps = []
    for c in range(NCORES):
        xcore = toks[c * CTOK:(c + 1) * CTOK]            # [2048, 1024]
        # xh[ch, p, k, w] = x[ch*512+w, k*128+p]
        xch = np.ascontiguousarray(
            xcore.reshape(CH, CW, KT, 128).transpose(0, 3, 2, 1)
        ).astype(NPBF16).reshape(CH, 128, KT * CW)
        in_maps.append({
            "xc": xch,
            "wt": wt,
            "atp": atp,
            "btp": btp,
            "biasc": biasc,
        })

    nc = _get_program()
    res = bass_utils.run_bass_kernel_spmd(nc, in_maps, core_ids=list(range(NCORES)))

    full = np.empty((T, NTOK, DOUT), np.float32)
    for c in range(NCORES):
        o = np.asarray(res.results[c]["out"])   # [CH, MT, 128, T*CW] bf16
        # o[ch, m, p, t, w] -> [t, ch*CW + w, m*128 + p]
        oc = o.reshape(CH, MT, 128, T, CW).transpose(3, 0, 4, 1, 2) \
              .reshape(T, CTOK, DOUT)
        full[:, c * CTOK:(c + 1) * CTOK, :] = oc.astype(np.float32)
    return full.reshape(T, B, S, DOUT)
